# revision 6
# baseline (speedup 1.0000x reference)
"""2D Haar DWT (grouped, stride-2 2x2 stencil) on 8 Trainium2 NeuronCores.

Contract: kernel(**inputs) takes FULL inputs (x: [8, 256, 224, 224] f32,
w_ll/w_lh/w_hl/w_hh: [2, 2] f32) and returns the FULL output
[8, 256, 112, 112] f32. Data-parallel over batch: sample i -> core i.

Layout (the key trick): SBUF partition = OUTPUT ROW i (112 of 128
partitions). Partition i holds input rows {2i, 2i+1} concatenated in the
free dimension, so the H-direction pairing, the W-direction pairing AND
the 4-channel group-sum are all free-dimension ops. Every DMA has a
plain single-stride partition dim — composite (partition-split) APs are
silently mis-executed by the HW DMA path and are avoided entirely.

Per core (x_c: [256, 224, 224] -> out_c: [256, 112, 112]), iterating
over channel blocks of CB=16 (4 groups) at a time:
  1. Load T[112, 16, 448] <- x[cblock, :, :] with one DMA:
     T[i, c, :] = concat(x[c, 2i, :], x[c, 2i+1, :]).
  2. Group-sum xs[112, 4, 448] = sum over the 4 sub-channels of each
     group (DVE pair-add + GPSIMD pair-add -> DVE merge).
  3. W-pairs: u = xs[..., 0::2] + xs[..., 1::2], v = same with minus
     -> [112, 4, 2, 112] = (group, row-parity rp, out-col j). Scale by
     the Haar lam in-place on the (idle) ScalarE.
  4. H-pairs (free dim!): ll = u[rp=0]+u[rp=1], lh = u0-u1,
     hl = v0+v1, hh = v0-v1 (hl/hh on GPSIMD to offload DVE).
  5. Store: out[s*64+g, i, :] one DMA per s per block.
Non-Haar weights fall back to a generic 4-term MAC per output.

Framework constraints baked in: TRN2 compute instructions carry ONE sync
wait (bacc hoists extras into EventSemaphore nops); Tile's release
tracking loses scalar_tensor_tensor's third operand (so it is not used);
pool slot recycling mis-syncs for this program shape (so every tile is
allocated once and ring-buffered manually).
"""

import numpy as np

B, C, H, W = 8, 256, 224, 224
G = C // 4                 # 64 groups
HO, WO = H // 2, W // 2    # 112, 112
N_CORES = 8
CB = 16                    # channels per iteration (4 groups)
GB = CB // 4               # groups per iteration
N_ITER = C // CB           # 16
P = HO                     # 112 partitions = output rows
W2 = 2 * W                 # row-pair concat length (448)

SIM_MEMSET = False  # debug harnesses only
STORE_ENGINE = "scalar"  # ACT HWDGE ring: keeps stores off the load (SP) FIFO
LOAD_ENGINE = "sync"    # sync | scalar | alt : DGE ring for loads
MUL_ENGINE = "vector"   # DVE tensor_scalar (2x mode) keeps ACT a pure store ring
NBUF = 4   # load-tile ring depth
RING = 2   # compute-tile ring depth
LOAD_PAIR = False  # one [112, 32ch] load DMA per TWO iterations (8 DMAs total)


def _haar_scale(w_ll, w_lh, w_hl, w_hh):
    """Return the common scale s^2 if the four 2x2 kernels are the
    standard Haar set (lo=[s,s], hi=[s,-s] outer products), else None."""
    lam = float(w_ll[0, 0])
    if lam == 0.0 or not np.isfinite(lam):
        return None
    pats = [
        (w_ll, [[1, 1], [1, 1]]),
        (w_lh, [[1, 1], [-1, -1]]),
        (w_hl, [[1, -1], [1, -1]]),
        (w_hh, [[1, -1], [-1, 1]]),
    ]
    for w, p in pats:
        if not np.allclose(np.asarray(w, np.float64) / lam, p, atol=1e-5):
            return None
    return lam


def _build(lam, ws, repeats=1):
    # repeats>1 wraps the body in a device-side For_i loop (benchmarking
    # only; keep repeats*N_ITER*DMAS*16 < 65536 to avoid sem overflow).
    import contextlib

    import concourse.bacc as bacc
    import concourse.mybir as mybir
    import concourse.tile as tile

    f32 = mybir.dt.float32
    ADD = mybir.AluOpType.add
    SUB = mybir.AluOpType.subtract
    MULT = mybir.AluOpType.mult

    nc = bacc.Bacc("TRN2", target_bir_lowering=False, debug=False)
    x = nc.declare_dram_parameter("x", [C, H, W], f32, isOutput=False)
    out = nc.declare_dram_parameter("out", [C, HO, WO], f32, isOutput=True)

    with tile.TileContext(nc) as tc:
        with tc.tile_pool(name="tp", bufs=1) as tp:
            # allocate all tiles once; ring-buffer manually (pool slot
            # recycling mis-syncs for this program shape)
            if LOAD_PAIR:
                # two iterations' channels per tile; loads drop 16 -> 8 DMAs
                # (amortizes per-DMA HWDGE ring serialization ~0.8us each)
                tp_bufs = [
                    tp.tile([P, 2 * CB, W2], f32, tag=f"tp{i}", name=f"tp{i}")
                    for i in range(2)
                ]
                t_bufs = None
            else:
                t_bufs = [
                    tp.tile([P, CB, W2], f32, tag=f"t{i}", name=f"t{i}")
                    for i in range(NBUF)
                ]
            s2_bufs = [
                tp.tile([P, 2, GB, W2], f32, tag=f"s2_{i}", name=f"s2_{i}")
                for i in range(RING)
            ]
            xs_bufs = [
                tp.tile([P, GB, W2], f32, tag=f"xs_{i}", name=f"xs_{i}")
                for i in range(RING)
            ]
            uv_bufs = [
                (
                    tp.tile([P, GB, 2, WO], f32, tag=f"u_{i}", name=f"u_{i}"),
                    tp.tile([P, GB, 2, WO], f32, tag=f"v_{i}", name=f"v_{i}"),
                )
                for i in range(RING)
            ]
            o_bufs = [
                [
                    tp.tile([P, GB, WO], f32, tag=f"o{s}_{i}", name=f"o{s}_{i}")
                    for s in range(4)
                ]
                for i in range(RING)
            ]
            if lam is None:
                q_bufs = [
                    tp.tile([P, 4, GB, WO], f32, tag=f"q_{i}", name=f"q_{i}")
                    for i in range(RING)
                ]
                p_bufs = [
                    tp.tile([P, 2, GB, WO], f32, tag=f"p_{i}", name=f"p_{i}")
                    for i in range(RING)
                ]
            else:
                q_bufs = p_bufs = []
            if SIM_MEMSET:
                for t in t_bufs + s2_bufs + xs_bufs + list(q_bufs) + list(p_bufs):
                    nc.vector.memset(t[:], 0.0)
                for a, b in uv_bufs:
                    nc.vector.memset(a[:], 0.0)
                    nc.vector.memset(b[:], 0.0)
                for four in o_bufs:
                    for tt in four:
                        nc.vector.memset(tt[:], 0.0)

            loop_ctx = (
                tc.For_i(0, repeats, 1)
                if repeats > 1
                else contextlib.nullcontext()
            )
            with loop_ctx:
              for it in range(N_ITER):
                c0 = it * CB

                # ---- load: one 112-partition DMA for the channel block ----
                # T[i, c, :] = x[c0+c, 2i:2i+2, :] flattened
                if LOAD_PAIR:
                    tpair = tp_bufs[(it // 2) % 2]
                    if it % 2 == 0:
                        src = (
                            x[c0 : c0 + 2 * CB, :, :]
                            .rearrange("c (i p) w -> c i p w", p=2)
                            .rearrange("c i p w -> i c (p w)")
                        )
                        getattr(nc, LOAD_ENGINE).dma_start(tpair[:], src)
                    t = tpair[:, (it % 2) * CB : (it % 2 + 1) * CB]
                else:
                    t = t_bufs[it % NBUF]
                    src = (
                        x[c0 : c0 + CB, :, :]
                        .rearrange("c (i p) w -> c i p w", p=2)
                        .rearrange("c i p w -> i c (p w)")
                    )
                    if LOAD_ENGINE == "alt":
                        ld_eng = nc.sync if it % 2 == 0 else nc.scalar
                    else:
                        ld_eng = getattr(nc, LOAD_ENGINE)
                    ld_eng.dma_start(t[:], src)

                # ---- group-sum over sub-channels (free dim) ----
                if LOAD_PAIR:
                    t4 = t.rearrange("i (g k) f -> i g k f", k=4)
                else:
                    t4 = t[:].rearrange("i (g k) f -> i g k f", k=4)
                s2 = s2_bufs[it % RING]
                nc.vector.tensor_tensor(s2[:, 0], t4[:, :, 0], t4[:, :, 1], ADD)
                nc.gpsimd.tensor_tensor(s2[:, 1], t4[:, :, 2], t4[:, :, 3], ADD)
                xs = xs_bufs[it % RING]
                nc.vector.tensor_tensor(xs[:], s2[:, 0], s2[:, 1], ADD)

                if lam is not None:
                    u, v = uv_bufs[it % RING]
                    uf = u[:].rearrange("i g p j -> i g (p j)")
                    vf = v[:].rearrange("i g p j -> i g (p j)")
                    # fold the global scale in-place; MUL_ENGINE picks
                    # ScalarE (idle ALU) or DVE tensor_scalar (2x mode,
                    # keeps the ACT sequencer free for the store ring);
                    # "xs_scalar" scales xs once on ACT before the W-pairs
                    # (one op instead of two, and zero DVE multiply work)
                    if MUL_ENGINE == "xs_scalar":
                        nc.scalar.mul(xs[:], xs[:], lam)
                    nc.vector.tensor_tensor(
                        uf, xs[:, :, 0::2], xs[:, :, 1::2], ADD
                    )
                    nc.vector.tensor_tensor(
                        vf, xs[:, :, 0::2], xs[:, :, 1::2], SUB
                    )
                    if MUL_ENGINE == "scalar":
                        nc.scalar.mul(u[:], u[:], lam)
                        nc.scalar.mul(v[:], v[:], lam)
                    elif MUL_ENGINE != "xs_scalar":
                        nc.vector.tensor_scalar(u[:], u[:], lam, None, MULT)
                        nc.vector.tensor_scalar(v[:], v[:], lam, None, MULT)
                    outs = o_bufs[it % RING]
                    nc.vector.tensor_tensor(
                        outs[0][:], u[:, :, 0], u[:, :, 1], ADD
                    )
                    nc.vector.tensor_tensor(
                        outs[1][:], u[:, :, 0], u[:, :, 1], SUB
                    )
                    nc.gpsimd.tensor_tensor(
                        outs[2][:], v[:, :, 0], v[:, :, 1], ADD
                    )
                    nc.gpsimd.tensor_tensor(
                        outs[3][:], v[:, :, 0], v[:, :, 1], SUB
                    )
                else:
                    # generic 2x2 stencil: out_s = sum coef * {a,b,c,d}
                    xs4 = xs[:].rearrange("i g (p w) -> i g p w", p=2)
                    abcd = [
                        xs4[:, :, 0, 0::2],
                        xs4[:, :, 0, 1::2],
                        xs4[:, :, 1, 0::2],
                        xs4[:, :, 1, 1::2],
                    ]
                    q = q_bufs[it % RING]
                    p2 = p_bufs[it % RING]
                    outs = o_bufs[it % RING]
                    for s in range(4):
                        w = ws[s]
                        coef = [
                            float(w[0, 0]),
                            float(w[0, 1]),
                            float(w[1, 0]),
                            float(w[1, 1]),
                        ]
                        for j in range(4):
                            nc.vector.tensor_scalar(
                                q[:, j], abcd[j], coef[j], None, MULT
                            )
                        nc.vector.tensor_tensor(p2[:, 0], q[:, 0], q[:, 1], ADD)
                        nc.vector.tensor_tensor(p2[:, 1], q[:, 2], q[:, 3], ADD)
                        nc.vector.tensor_tensor(
                            outs[s][:], p2[:, 0], p2[:, 1], ADD
                        )

                # ---- store: one DMA per output s ----
                g0 = it * GB
                if STORE_ENGINE == "alt":
                    st_eng = nc.scalar if it % 2 == 0 else nc.sync
                else:
                    st_eng = getattr(nc, STORE_ENGINE)
                for s in range(4):
                    dst = out[s * G + g0 : s * G + g0 + GB, :, :].rearrange(
                        "c i j -> i c j"
                    )
                    st_eng.dma_start(dst, outs[s][:])
    nc.finalize()  # run Bacc passes (regalloc, nop->event wait legalization)
    return nc


def kernel(x, w_ll, w_lh, w_hl, w_hh):
    from concourse.bass_utils import run_bass_kernel_spmd

    x = np.ascontiguousarray(np.asarray(x, dtype=np.float32))
    assert x.shape == (B, C, H, W), x.shape
    ws = [
        np.asarray(w, dtype=np.float32) for w in (w_ll, w_lh, w_hl, w_hh)
    ]
    lam = _haar_scale(*ws)
    nc = _build(lam, ws)
    in_maps = [{"x": x[i]} for i in range(N_CORES)]
    r = run_bass_kernel_spmd(nc, in_maps, list(range(N_CORES)))
    return np.stack([r.results[i]["out"] for i in range(N_CORES)], axis=0)



# revision 8
# speedup vs baseline: 1.0573x; 1.0573x over previous
"""2D Haar DWT (grouped, stride-2 2x2 stencil) on 8 Trainium2 NeuronCores.

Contract: kernel(**inputs) takes FULL inputs (x: [8, 256, 224, 224] f32,
w_ll/w_lh/w_hl/w_hh: [2, 2] f32) and returns the FULL output
[8, 256, 112, 112] f32. Data-parallel over batch: sample i -> core i.

Layout (the key trick): SBUF partition = OUTPUT ROW i (112 of 128
partitions). Partition i holds input rows {2i, 2i+1} concatenated in the
free dimension, so the H-direction pairing, the W-direction pairing AND
the 4-channel group-sum are all free-dimension ops. Every DMA has a
plain single-stride partition dim — composite (partition-split) APs are
silently mis-executed by the HW DMA path and are avoided entirely.

Per core (x_c: [256, 224, 224] -> out_c: [256, 112, 112]), iterating
over channel blocks of CB=16 (4 groups) at a time:
  1. Load T[112, 16, 448] <- x[cblock, :, :] with one DMA:
     T[i, c, :] = concat(x[c, 2i, :], x[c, 2i+1, :]).
  2. Group-sum xs[112, 4, 448] = sum over the 4 sub-channels of each
     group (DVE pair-add + GPSIMD pair-add -> DVE merge).
  3. W-pairs: u = xs[..., 0::2] + xs[..., 1::2], v = same with minus
     -> [112, 4, 2, 112] = (group, row-parity rp, out-col j). Scale by
     the Haar lam in-place on the (idle) ScalarE.
  4. H-pairs (free dim!): ll = u[rp=0]+u[rp=1], lh = u0-u1,
     hl = v0+v1, hh = v0-v1 (hl/hh on GPSIMD to offload DVE).
  5. Store: out[s*64+g, i, :] one DMA per s per block.
Non-Haar weights fall back to a generic 4-term MAC per output.

Framework constraints baked in: TRN2 compute instructions carry ONE sync
wait (bacc hoists extras into EventSemaphore nops); Tile's release
tracking loses scalar_tensor_tensor's third operand (so it is not used);
pool slot recycling mis-syncs for this program shape (so every tile is
allocated once and ring-buffered manually).
"""

import numpy as np

B, C, H, W = 8, 256, 224, 224
G = C // 4                 # 64 groups
HO, WO = H // 2, W // 2    # 112, 112
N_CORES = 8
CB = 16                    # channels per iteration (4 groups)
GB = CB // 4               # groups per iteration
N_ITER = C // CB           # 16
P = HO                     # 112 partitions = output rows
W2 = 2 * W                 # row-pair concat length (448)

SIM_MEMSET = False  # debug harnesses only
STORE_ENGINE = "scalar"  # ACT HWDGE ring: keeps stores off the load (SP) FIFO
LOAD_ENGINE = "sync"    # sync | scalar | alt : DGE ring for loads
MUL_ENGINE = "vector"   # DVE tensor_scalar (2x mode) keeps ACT a pure store ring
NBUF = 4   # load-tile ring depth
RING = 2   # compute-tile ring depth
LOAD_PAIR = False  # one [112, 32ch] load DMA per TWO iterations (8 DMAs total)
STORE_SINGLE_PACKET = False  # single_packet on store DMAs (448B descs)


def _haar_scale(w_ll, w_lh, w_hl, w_hh):
    """Return the common scale s^2 if the four 2x2 kernels are the
    standard Haar set (lo=[s,s], hi=[s,-s] outer products), else None."""
    lam = float(w_ll[0, 0])
    if lam == 0.0 or not np.isfinite(lam):
        return None
    pats = [
        (w_ll, [[1, 1], [1, 1]]),
        (w_lh, [[1, 1], [-1, -1]]),
        (w_hl, [[1, -1], [1, -1]]),
        (w_hh, [[1, -1], [-1, 1]]),
    ]
    for w, p in pats:
        if not np.allclose(np.asarray(w, np.float64) / lam, p, atol=1e-5):
            return None
    return lam


def _build(lam, ws, repeats=1):
    # repeats>1 wraps the body in a device-side For_i loop (benchmarking
    # only; keep repeats*N_ITER*DMAS*16 < 65536 to avoid sem overflow).
    import contextlib

    import concourse.bacc as bacc
    import concourse.mybir as mybir
    import concourse.tile as tile

    f32 = mybir.dt.float32
    ADD = mybir.AluOpType.add
    SUB = mybir.AluOpType.subtract
    MULT = mybir.AluOpType.mult

    nc = bacc.Bacc("TRN2", target_bir_lowering=False, debug=False)
    x = nc.declare_dram_parameter("x", [C, H, W], f32, isOutput=False)
    out = nc.declare_dram_parameter("out", [C, HO, WO], f32, isOutput=True)

    with tile.TileContext(nc) as tc:
        with tc.tile_pool(name="tp", bufs=1) as tp:
            # allocate all tiles once; ring-buffer manually (pool slot
            # recycling mis-syncs for this program shape)
            if LOAD_PAIR:
                # two iterations' channels per tile; loads drop 16 -> 8 DMAs
                # (amortizes per-DMA HWDGE ring serialization ~0.8us each)
                tp_bufs = [
                    tp.tile([P, 2 * CB, W2], f32, tag=f"tp{i}", name=f"tp{i}")
                    for i in range(2)
                ]
                t_bufs = None
            else:
                t_bufs = [
                    tp.tile([P, CB, W2], f32, tag=f"t{i}", name=f"t{i}")
                    for i in range(NBUF)
                ]
            s2_bufs = [
                tp.tile([P, 2, GB, W2], f32, tag=f"s2_{i}", name=f"s2_{i}")
                for i in range(RING)
            ]
            xs_bufs = [
                tp.tile([P, GB, W2], f32, tag=f"xs_{i}", name=f"xs_{i}")
                for i in range(RING)
            ]
            uv_bufs = [
                (
                    tp.tile([P, GB, 2, WO], f32, tag=f"u_{i}", name=f"u_{i}"),
                    tp.tile([P, GB, 2, WO], f32, tag=f"v_{i}", name=f"v_{i}"),
                )
                for i in range(RING)
            ]
            o_bufs = [
                [
                    tp.tile([P, GB, WO], f32, tag=f"o{s}_{i}", name=f"o{s}_{i}")
                    for s in range(4)
                ]
                for i in range(RING)
            ]
            if lam is None:
                q_bufs = [
                    tp.tile([P, 4, GB, WO], f32, tag=f"q_{i}", name=f"q_{i}")
                    for i in range(RING)
                ]
                p_bufs = [
                    tp.tile([P, 2, GB, WO], f32, tag=f"p_{i}", name=f"p_{i}")
                    for i in range(RING)
                ]
            else:
                q_bufs = p_bufs = []
            if SIM_MEMSET:
                for t in t_bufs + s2_bufs + xs_bufs + list(q_bufs) + list(p_bufs):
                    nc.vector.memset(t[:], 0.0)
                for a, b in uv_bufs:
                    nc.vector.memset(a[:], 0.0)
                    nc.vector.memset(b[:], 0.0)
                for four in o_bufs:
                    for tt in four:
                        nc.vector.memset(tt[:], 0.0)

            loop_ctx = (
                tc.For_i(0, repeats, 1)
                if repeats > 1
                else contextlib.nullcontext()
            )
            with loop_ctx:
              for it in range(N_ITER):
                c0 = it * CB

                # ---- load: one 112-partition DMA for the channel block ----
                # T[i, c, :] = x[c0+c, 2i:2i+2, :] flattened
                if LOAD_PAIR:
                    tpair = tp_bufs[(it // 2) % 2]
                    if it % 2 == 0:
                        src = (
                            x[c0 : c0 + 2 * CB, :, :]
                            .rearrange("c (i p) w -> c i p w", p=2)
                            .rearrange("c i p w -> i c (p w)")
                        )
                        getattr(nc, LOAD_ENGINE).dma_start(tpair[:], src)
                    t = tpair[:, (it % 2) * CB : (it % 2 + 1) * CB]
                else:
                    t = t_bufs[it % NBUF]
                    src = (
                        x[c0 : c0 + CB, :, :]
                        .rearrange("c (i p) w -> c i p w", p=2)
                        .rearrange("c i p w -> i c (p w)")
                    )
                    if LOAD_ENGINE == "alt":
                        ld_eng = nc.sync if it % 2 == 0 else nc.scalar
                    else:
                        ld_eng = getattr(nc, LOAD_ENGINE)
                    ld_eng.dma_start(t[:], src)

                # ---- group-sum over sub-channels (free dim) ----
                if LOAD_PAIR:
                    t4 = t.rearrange("i (g k) f -> i g k f", k=4)
                else:
                    t4 = t[:].rearrange("i (g k) f -> i g k f", k=4)
                s2 = s2_bufs[it % RING]
                nc.vector.tensor_tensor(s2[:, 0], t4[:, :, 0], t4[:, :, 1], ADD)
                nc.gpsimd.tensor_tensor(s2[:, 1], t4[:, :, 2], t4[:, :, 3], ADD)
                xs = xs_bufs[it % RING]
                nc.vector.tensor_tensor(xs[:], s2[:, 0], s2[:, 1], ADD)

                if lam is not None:
                    u, v = uv_bufs[it % RING]
                    uf = u[:].rearrange("i g p j -> i g (p j)")
                    vf = v[:].rearrange("i g p j -> i g (p j)")
                    # fold the global scale in-place; MUL_ENGINE picks
                    # ScalarE (idle ALU) or DVE tensor_scalar (2x mode,
                    # keeps the ACT sequencer free for the store ring);
                    # "xs_scalar" scales xs once on ACT before the W-pairs
                    # (one op instead of two, and zero DVE multiply work)
                    if MUL_ENGINE == "xs_scalar":
                        nc.scalar.mul(xs[:], xs[:], lam)
                    nc.vector.tensor_tensor(
                        uf, xs[:, :, 0::2], xs[:, :, 1::2], ADD
                    )
                    nc.vector.tensor_tensor(
                        vf, xs[:, :, 0::2], xs[:, :, 1::2], SUB
                    )
                    if MUL_ENGINE == "scalar":
                        nc.scalar.mul(u[:], u[:], lam)
                        nc.scalar.mul(v[:], v[:], lam)
                    elif MUL_ENGINE != "xs_scalar":
                        nc.vector.tensor_scalar(u[:], u[:], lam, None, MULT)
                        nc.vector.tensor_scalar(v[:], v[:], lam, None, MULT)
                    outs = o_bufs[it % RING]
                    nc.vector.tensor_tensor(
                        outs[0][:], u[:, :, 0], u[:, :, 1], ADD
                    )
                    nc.vector.tensor_tensor(
                        outs[1][:], u[:, :, 0], u[:, :, 1], SUB
                    )
                    nc.gpsimd.tensor_tensor(
                        outs[2][:], v[:, :, 0], v[:, :, 1], ADD
                    )
                    nc.gpsimd.tensor_tensor(
                        outs[3][:], v[:, :, 0], v[:, :, 1], SUB
                    )
                else:
                    # generic 2x2 stencil: out_s = sum coef * {a,b,c,d}
                    xs4 = xs[:].rearrange("i g (p w) -> i g p w", p=2)
                    abcd = [
                        xs4[:, :, 0, 0::2],
                        xs4[:, :, 0, 1::2],
                        xs4[:, :, 1, 0::2],
                        xs4[:, :, 1, 1::2],
                    ]
                    q = q_bufs[it % RING]
                    p2 = p_bufs[it % RING]
                    outs = o_bufs[it % RING]
                    for s in range(4):
                        w = ws[s]
                        coef = [
                            float(w[0, 0]),
                            float(w[0, 1]),
                            float(w[1, 0]),
                            float(w[1, 1]),
                        ]
                        for j in range(4):
                            nc.vector.tensor_scalar(
                                q[:, j], abcd[j], coef[j], None, MULT
                            )
                        nc.vector.tensor_tensor(p2[:, 0], q[:, 0], q[:, 1], ADD)
                        nc.vector.tensor_tensor(p2[:, 1], q[:, 2], q[:, 3], ADD)
                        nc.vector.tensor_tensor(
                            outs[s][:], p2[:, 0], p2[:, 1], ADD
                        )

                # ---- store: one DMA per output s ----
                g0 = it * GB
                if STORE_ENGINE == "alt":
                    st_eng = nc.scalar if it % 2 == 0 else nc.sync
                else:
                    st_eng = getattr(nc, STORE_ENGINE)
                for s in range(4):
                    dst = out[s * G + g0 : s * G + g0 + GB, :, :].rearrange(
                        "c i j -> i c j"
                    )
                    st_eng.dma_start(
                        dst, outs[s][:], single_packet=STORE_SINGLE_PACKET
                    )
    nc.finalize()  # run Bacc passes (regalloc, nop->event wait legalization)
    return nc


def kernel(x, w_ll, w_lh, w_hl, w_hh):
    from concourse.bass_utils import run_bass_kernel_spmd

    x = np.ascontiguousarray(np.asarray(x, dtype=np.float32))
    assert x.shape == (B, C, H, W), x.shape
    ws = [
        np.asarray(w, dtype=np.float32) for w in (w_ll, w_lh, w_hl, w_hh)
    ]
    lam = _haar_scale(*ws)
    nc = _build(lam, ws)
    in_maps = [{"x": x[i]} for i in range(N_CORES)]
    r = run_bass_kernel_spmd(nc, in_maps, list(range(N_CORES)))
    return np.stack([r.results[i]["out"] for i in range(N_CORES)], axis=0)

